# revision 13
# baseline (speedup 1.0000x reference)
"""Cross-attention kernel for 8 TRN2 NeuronCores.

Problem: B=4, T_V=8192, T_T=77, C=1024, H=16, D=64 (f32).
  q = video @ Wq.T ; k,v = text @ W.T ; out = softmax(qk/sqrt(D)) v @ Wo.T

Sharding: data-parallel over (batch, T_V/2) -> 8 shards of [4096, 1024].
Each core gets its video shard, its batch's text, and all weights.
No collectives.

On-chip dataflow (everything "transposed": rows of video on the FREE dim):
  host pre-transposes X -> X^T [C, M] and weights -> W^T [C, C] so the
  contraction dim always lands on SBUF partitions.
  Q^T = WqT-chunks . X^T          (f32r matmuls, N=512 -> full PE rate)
  K^T [C, T] from text, V natural [T, C]
  per head h: S^T = K_h^T . Q_h^T -> exp on ScalarE (scale=1/8 folded in,
  no max-subtraction: scores are O(1) bounded).
  softmax denominator: GPSIMD partition_all_reduce over the 77 key
  partitions of expS^T (idle Pool engine, parallel with the AV matmul);
  result is replicated across partitions. Heads are processed in pairs
  sharing one [128,512] PSUM tile and one [77,2,512] denominator tile,
  so normalization is one DVE approx-reciprocal + one DVE multiply per
  PAIR. No broadcast DMAs, no single-lane reciprocals.
  out = O^T-chunks . WoT in natural [m, n] layout (PSUM -> SBUF on
  ScalarE, then DMA).
  Software pipelining: per iteration emit [xt-dma_j, Q_j, out_{j-1},
  attn_j] so attention post-ops drain while PE runs the next dense
  GEMMs, and out-proj never waits on normalization.
"""

import sys

if "/opt/trn_rl_repo" not in sys.path:
    sys.path.insert(0, "/opt/trn_rl_repo")

import numpy as np

import concourse.bacc as bacc
import concourse.bass as bass
import concourse.mybir as mybir
import concourse.tile as tile
from concourse.bass_utils import run_bass_kernel_spmd

F32 = mybir.dt.float32
F32R = mybir.dt.float32r
BF16 = mybir.dt.bfloat16
AF = mybir.ActivationFunctionType
ALU = mybir.AluOpType
RED = bass.bass_isa.ReduceOp

B, T_V, T_T, C, H = 4, 8192, 77, 1024, 16
D = C // H            # 64
P = 128
KC = C // P           # 8 contraction chunks
M = T_V // 2          # 4096 rows per core
MB = 512              # m-block (rows processed per pipeline stage)
NBLK = M // MB        # 8
MSUB = MB // P        # 4 output row-chunks per block
T = T_T               # 77
TP = 80               # padded T for even-moving-dim f32r matmuls
SCALE = 1.0 / float(np.sqrt(D))

_CACHED_NC = None
DEBUG = False
ABLATE = None  # None | 'allred' | 'norm' | 'exp'


def _build(repeat: int = 1):
    nc = bacc.Bacc(name="cross_attention")

    xt = nc.dram_tensor("xt", [C, M], BF16, kind="ExternalInput")
    yt = nc.dram_tensor("yt", [C, T], BF16, kind="ExternalInput")
    wqt = nc.dram_tensor("wqt", [C, C], BF16, kind="ExternalInput")
    wkt = nc.dram_tensor("wkt", [C, C], BF16, kind="ExternalInput")
    wvt = nc.dram_tensor("wvt", [C, C], BF16, kind="ExternalInput")
    wot = nc.dram_tensor("wot", [C, C], BF16, kind="ExternalInput")
    out = nc.dram_tensor("out", [M, C], F32, kind="ExternalOutput")
    dbg = {}
    if DEBUG:
        dbg["kt"] = nc.dram_tensor("dbg_kt", [P, KC, T], BF16, kind="ExternalOutput")
        dbg["v"] = nc.dram_tensor("dbg_v", [T, H, D], BF16, kind="ExternalOutput")
        for j in (0, 1):
            dbg[f"qt{j}"] = nc.dram_tensor(f"dbg_qt{j}", [P, KC, MB], BF16, kind="ExternalOutput")
            dbg[f"ot{j}"] = nc.dram_tensor(f"dbg_ot{j}", [P, KC, MB], BF16, kind="ExternalOutput")
            for h in (0, 3):
                dbg[f"es{j}_{h}"] = nc.dram_tensor(f"dbg_es{j}_{h}", [T, MB], BF16, kind="ExternalOutput")
            dbg[f"rb{j}"] = nc.dram_tensor(f"dbg_rb{j}", [T, MB], F32, kind="ExternalOutput")
            dbg[f"rr{j}"] = nc.dram_tensor(f"dbg_rr{j}", [P, MB], F32, kind="ExternalOutput")

    # [C, X] dram views chunked to [P, KC, X]
    xt_v = xt[:, :].rearrange("(kc p) m -> p kc m", p=P)
    yt_v = yt[:, :].rearrange("(kc p) t -> p kc t", p=P)
    wq_v = wqt[:, :].rearrange("(kc p) n -> p kc n", p=P)
    wk_v = wkt[:, :].rearrange("(kc p) n -> p kc n", p=P)
    wv_v = wvt[:, :].rearrange("(kc p) n -> p kc n", p=P)
    wo_v = wot[:, :].rearrange("(kc p) n -> p kc n", p=P)

    with tile.TileContext(nc) as tc:
        with (
            tc.tile_pool(name="wq", bufs=1) as wq_pool,
            tc.tile_pool(name="wo", bufs=1) as wo_pool,
            tc.tile_pool(name="kt", bufs=1) as kt_pool,
            tc.tile_pool(name="vv", bufs=1) as v_pool,
            tc.tile_pool(name="wkv", bufs=1) as wkv_pool,
            tc.tile_pool(name="yt", bufs=1) as yt_pool,
            tc.tile_pool(name="xt", bufs=2) as xt_pool,
            tc.tile_pool(name="qt", bufs=2) as qt_pool,
            tc.tile_pool(name="ot", bufs=2) as ot_pool,
            tc.tile_pool(name="es", bufs=3) as es_pool,
            tc.tile_pool(name="rb", bufs=3) as rb_pool,
            tc.tile_pool(name="rr", bufs=2) as rr_pool,
            tc.tile_pool(name="ob", bufs=3) as ob_pool,
            tc.tile_pool(name="psmm", bufs=2, space="PSUM") as ps_mm,
            tc.tile_pool(name="pss", bufs=3, space="PSUM") as ps_s,
            tc.tile_pool(name="pso", bufs=3, space="PSUM") as ps_o,
        ):
            wq_sb = wq_pool.tile([P, KC, C], BF16)
            wo_sb = wo_pool.tile([P, KC, C], BF16)
            kt_sb = kt_pool.tile([P, KC, T], BF16)
            v_sb = v_pool.tile([T, H, D], BF16)

            rbc = None
            if ABLATE == "allred":
                rbc = rb_pool.tile([T, 2, MB], F32, tag="rbc")
                nc.vector.memset(rbc[:], 1.0)

            xt_tiles = {}

            def emit_xt_dma(j):
                xt_t = xt_pool.tile([P, KC, MB], BF16, tag="xt")
                xt_tiles[j] = xt_t
                nc.sync.dma_start(xt_t[:], xt_v[:, :, j * MB : (j + 1) * MB])

            qt_tiles = {}

            def emit_qproj(j):
                qt_t = qt_pool.tile([P, KC, MB], BF16, tag="qt")
                qt_tiles[j] = qt_t
                xt_t = xt_tiles.pop(j)
                for nc_ in range(KC):
                    psq = ps_mm.tile([P, MB], F32, tag="mm")
                    for kc in range(KC):
                        nc.tensor.matmul(
                            psq[:],
                            wq_sb[:, kc, nc_ * P : (nc_ + 1) * P],
                            xt_t[:, kc, :],
                            start=(kc == 0),
                            stop=(kc == KC - 1),
                        )
                    nc.scalar.copy(out=qt_t[:, nc_, :], in_=psq[:])
                if DEBUG and j in (0, 1):
                    nc.sync.dma_start(dbg[f"qt{j}"][:, :, :], qt_t[:])

            ot_tiles = {}

            def emit_attn(j):
                qt_t = qt_tiles.pop(j)
                ot_t = ot_pool.tile([P, KC, MB], BF16, tag="ot")
                ot_tiles[j] = ot_t
                for jc in range(KC):  # head pair (2*jc, 2*jc+1)
                    pso = ps_o.tile([P, MB], F32, tag="pso")
                    rb = rbc if ABLATE == "allred" else rb_pool.tile([T, 2, MB], F32, tag="rb")
                    for hf in range(2):
                        h = 2 * jc + hf
                        lo, hi = 64 * hf, 64 * hf + 64
                        pss = ps_s.tile([T, MB], F32, tag="pss")
                        nc.tensor.matmul(
                            pss[:],
                            kt_sb[lo:hi, jc, :],
                            qt_t[lo:hi, jc, :],
                            start=True,
                            stop=True,
                        )
                        es = es_pool.tile([T, MB], BF16, tag="es")
                        if ABLATE == "exp":
                            nc.vector.tensor_copy(out=es[:], in_=pss[:])
                        else:
                            nc.scalar.activation(es[:], pss[:], AF.Exp, scale=SCALE)
                        if DEBUG and j in (0, 1) and h in (0, 3):
                            nc.sync.dma_start(dbg[f"es{j}_{h}"][:, :], es[:])
                        # denominator: sum expS over the 77 key partitions,
                        # replicated to all 77 rows (GPSIMD, off PE's path)
                        if ABLATE != "allred":
                            nc.gpsimd.partition_all_reduce(
                                rb[:, hf, :], es[:], T, RED.add
                            )
                        nc.tensor.matmul(
                            pso[lo:hi, :], v_sb[:, h, :], es[:],
                            start=True, stop=True,
                        )
                    if DEBUG and j in (0, 1) and jc == 0:
                        nc.sync.dma_start(dbg[f"rb{j}"][:, :], rb[:, 0, :])
                    # one approx-reciprocal for the pair (offset-0 APs only:
                    # the custom DVE uop mishandles partition offsets), then
                    # per-head multiplies with partition-base-mixed operands
                    rr = rr_pool.tile([D, 2, MB], F32, tag="rr")
                    if ABLATE != "norm":
                        nc.vector.reciprocal_approx_fast(rr[:], rb[0:D, :, :])
                    if DEBUG and j in (0, 1) and jc == 0:
                        nc.sync.dma_start(dbg[f"rr{j}"][0:D, :], rr[:, 0, :])
                        nc.sync.dma_start(dbg[f"rr{j}"][D:P, :], rr[:, 1, :])
                    if ABLATE == "norm":
                        nc.vector.tensor_copy(out=ot_t[:, jc, :], in_=pso[:])
                    else:
                        for hf in range(2):
                            lo, hi = 64 * hf, 64 * hf + 64
                            nc.vector.tensor_tensor(
                                ot_t[lo:hi, jc, :], pso[lo:hi, :],
                                rr[:, hf, :], ALU.mult,
                            )
                if DEBUG and j in (0, 1):
                    nc.sync.dma_start(dbg[f"ot{j}"][:, :, :], ot_t[:])

            def emit_outproj(j):
                ot_t = ot_tiles.pop(j)
                for mi in range(MSUB):
                    for nh in range(2):
                        pst = ps_mm.tile([P, MB], F32, tag="mm")
                        for cc in range(KC):
                            nc.tensor.matmul(
                                pst[:],
                                ot_t[:, cc, mi * P : (mi + 1) * P],
                                wo_sb[:, cc, nh * MB : (nh + 1) * MB],
                                start=(cc == 0),
                                stop=(cc == KC - 1),
                            )
                        ob = ob_pool.tile([P, MB], F32, tag="ob")
                        nc.scalar.copy(out=ob[:], in_=pst[:])
                        nc.sync.dma_start(
                            out[
                                j * MB + mi * P : j * MB + (mi + 1) * P,
                                nh * MB : (nh + 1) * MB,
                            ],
                            ob[:],
                        )

            # ---- prologue: overlap weight DMAs with first-block compute ----
            blocks = [jj for _ in range(repeat) for jj in range(NBLK)]

            emit_xt_dma(blocks[0])
            yt_sb = yt_pool.tile([P, KC, TP], BF16)
            nc.vector.memset(yt_sb[:], 0.0)
            nc.sync.dma_start(yt_sb[:, :, :T], yt_v[:])
            for kc in range(KC):
                nc.sync.dma_start(wq_sb[:, kc, :], wq_v[:, kc, :])

            emit_qproj(blocks[0])

            wk_sb = wkv_pool.tile([P, KC, C], BF16, tag="wkv")
            for kc in range(KC):
                nc.sync.dma_start(wk_sb[:, kc, :], wk_v[:, kc, :])
            # K^T [C, T]: chunk nc_ holds rows 128*nc_..128*nc_+128
            for nc_ in range(KC):
                psk_full = ps_mm.tile([P, MB], F32, tag="mm", name="psk")
                psk = psk_full[:, :TP]
                for kc in range(KC):
                    nc.tensor.matmul(
                        psk[:],
                        wk_sb[:, kc, nc_ * P : (nc_ + 1) * P],
                        yt_sb[:, kc, :],
                        start=(kc == 0),
                        stop=(kc == KC - 1),
                    )
                nc.vector.tensor_copy(out=kt_sb[:, nc_, :], in_=psk[:, :T])

            if len(blocks) > 1:
                emit_xt_dma(blocks[1])

            wv_sb = wkv_pool.tile([P, KC, C], BF16, tag="wkv")
            for kc in range(KC):
                nc.sync.dma_start(wv_sb[:, kc, :], wv_v[:, kc, :])
            # V natural [T, C] written per 512-wide column slab into
            # the strided per-head layout v_sb[t, h, 0:64]
            for half in range(2):
                psv_full = ps_mm.tile([P, MB], F32, tag="mm", name="psv")
                psv = psv_full[:T, :]
                for kc in range(KC):
                    nc.tensor.matmul(
                        psv[:],
                        yt_sb[:, kc, :T],
                        wv_sb[:, kc, half * MB : (half + 1) * MB],
                        start=(kc == 0),
                        stop=(kc == KC - 1),
                    )
                nc.vector.tensor_copy(
                    out=v_sb[:, half * 8 : (half + 1) * 8, :],
                    in_=psv[:].rearrange("t (h d) -> t h d", d=D),
                )

            for kc in range(KC):
                nc.sync.dma_start(wo_sb[:, kc, :], wo_v[:, kc, :])
            if DEBUG:
                nc.sync.dma_start(dbg["kt"][:, :, :], kt_sb[:])
                nc.sync.dma_start(dbg["v"][:, :, :], v_sb[:])

            # ---- software-pipelined main loop ----
            emit_attn(blocks[0])
            for i in range(1, len(blocks)):
                if i + 1 < len(blocks):
                    emit_xt_dma(blocks[i + 1])
                emit_qproj(blocks[i])
                emit_outproj(blocks[i - 1])
                emit_attn(blocks[i])
            emit_outproj(blocks[-1])
    nc.finalize()
    return nc


def _get_nc(repeat: int = 1):
    global _CACHED_NC
    if _CACHED_NC is None:
        _CACHED_NC = {}
    if repeat not in _CACHED_NC:
        _CACHED_NC[repeat] = _build(repeat)
    return _CACHED_NC[repeat]


def kernel(video_features, text_features, Wq, Wk, Wv, Wo, **_unused):
    import ml_dtypes

    bf16 = ml_dtypes.bfloat16
    video_features = np.asarray(video_features, dtype=np.float32)
    text_features = np.asarray(text_features, dtype=np.float32)
    wqt = np.ascontiguousarray(np.asarray(Wq, dtype=np.float32).T).astype(bf16)
    wkt = np.ascontiguousarray(np.asarray(Wk, dtype=np.float32).T).astype(bf16)
    wvt = np.ascontiguousarray(np.asarray(Wv, dtype=np.float32).T).astype(bf16)
    wot = np.ascontiguousarray(np.asarray(Wo, dtype=np.float32).T).astype(bf16)

    in_maps = []
    for c in range(8):
        b, half = divmod(c, 2)
        xs = video_features[b, half * M : (half + 1) * M, :]  # [M, C]
        in_maps.append(
            {
                "xt": np.ascontiguousarray(xs.T).astype(bf16),   # [C, M]
                "yt": np.ascontiguousarray(text_features[b].T).astype(bf16),
                "wqt": wqt,
                "wkt": wkt,
                "wvt": wvt,
                "wot": wot,
            }
        )

    res = run_bass_kernel_spmd(_get_nc(), in_maps, core_ids=list(range(8)))
    outf = np.empty((B, T_V, C), dtype=np.float32)
    for c in range(8):
        b, half = divmod(c, 2)
        outf[b, half * M : (half + 1) * M, :] = res.results[c]["out"]
    return outf
